# revision 1
# baseline (speedup 1.0000x reference)
"""CFNet interaction block on 8 trn2 NeuronCores — v2 SPMD bass/tile kernel.

Strategy (per core c of 8, SPMD — one program, per-core input data):
  - Edges are sharded by ATOM ranges: core c owns atoms [c*NA, (c+1)*NA) and
    all edges whose (sorted) seg_i falls in that range.  Output slices are
    disjoint; the host concatenates them (no device collective).
  - Within a core, edges are grouped by 128-atom chunk of seg_i (order
    preserved within a chunk) and each chunk is padded with dump edges to a
    uniform TPC tiles (TPC = max over cores/chunks, data-derived).  Dump
    edges have dijk = 0 -> w = ssp(ssp(0)) = 0 exactly, so they contribute
    nothing to any sum.
  - dijk is uploaded PRE-TRANSPOSED [n_in, E_PAD] in bf16: mm1 consumes the
    k-major tiles directly (no PE transposes, half the HBM bytes of fp32).
  - Per 512-edge block: mm1 (t1^T [f1,e] = W1^T dijk^T), ssp via Exp then
    Ln(0.5 + 0.5 e) (exactly softplus - log2), mm2 per 128-edge tile
    (z2 [e,f2]), ssp -> w, wf = w * f (DVE, bf16), one-hot S_t generated
    on-device (iota == cid), and convT[f,a] += wf_t^T @ S_t accumulated in
    PSUM across the chunk's TPC tiles (start/stop flags).
  - f = xf[idx_j] where xf = x @ Win is computed on-device into DRAM
    (bf16), then gathered per 4096-edge group with one SWDGE dma_gather of
    PAIR rows (512B = atoms 2r,2r+1; idx = idx_j>>1 fits int16), queues
    round-robined 0..3; the odd/even half is selected per tile on DVE with
    a parity mask before wf = w * f.
  - Tail per chunk (pipelined with edge phase): z3^T = Wout^T convT, ssp,
    v = h @ Wd, y = x + v.
"""

import math
import sys

import numpy as np
import ml_dtypes

sys.path.insert(0, "/opt/trn_rl_repo")

import concourse.bacc as bacc
import concourse.bass as bass
import concourse.mybir as mybir
from concourse import tile
from concourse.bass_utils import run_bass_kernel_spmd

dt = mybir.dt
AF = mybir.ActivationFunctionType
ALU = mybir.AluOpType
BF16 = ml_dtypes.bfloat16

N_CORES = 8
ACH = 128            # atoms per conv chunk
LOAD_E = 4096        # edges per dijk load group
GATH_E = 1024        # edges per dma_gather call (= SWDGE ring size)


def _ceil(a, b):
    return -(-a // b)


class Plan:
    def __init__(self, n_atoms, n_edges, n_in, tpc):
        assert n_atoms % N_CORES == 0
        self.n_atoms, self.n_edges, self.n_in = n_atoms, n_edges, n_in
        self.NA = n_atoms // N_CORES
        self.NCH = _ceil(self.NA, ACH)
        self.NA_PAD = self.NCH * ACH
        self.TPC = tpc
        self.T = self.NCH * tpc
        self.NQUAD = _ceil(self.T * 128, LOAD_E)
        self.E_PAD = self.NQUAD * LOAD_E
        self.NGC = self.E_PAD // GATH_E          # idx table rows
        self.NGC_RUN = _ceil(self.T * 128, GATH_E)  # gather calls issued
        self.NBLK = _ceil(self.T, 4)
        self.NFULL = n_in // 128                 # full 128-row k chunks
        self.KREM = n_in - self.NFULL * 128
        # xf table: atoms padded to 512
        self.NX_PAD = _ceil(n_atoms, 512) * 512
        self.NSG = self.NX_PAD // 512
        self.NPAIR = self.NX_PAD // 2
        assert self.NPAIR <= 32767


def shard_inputs(p, x, dijk, idx_j, seg_i):
    """Host-side preprocessing. Returns (common dict, per-core dicts)."""
    n_atoms, n_in = p.n_atoms, p.n_in
    idx_j = np.asarray(idx_j).astype(np.int64)
    seg_i = np.asarray(seg_i).astype(np.int64)
    bounds = np.searchsorted(seg_i, np.arange(N_CORES + 1) * p.NA)

    per_core = []
    for c in range(N_CORES):
        lo, hi = int(bounds[c]), int(bounds[c + 1])
        es = seg_i[lo:hi] - c * p.NA          # local atom ids [0, NA)
        ej = idx_j[lo:hi]
        ch = es // ACH                        # local chunk id
        cnt = np.bincount(ch, minlength=p.NCH)
        assert cnt.max(initial=0) <= p.TPC * 128

        dsh = np.zeros((p.E_PAD, n_in), dtype=BF16)
        cidv = np.zeros(p.E_PAD, dtype=np.int64)
        idxv = np.zeros(p.E_PAD, dtype=np.int64)
        starts = np.cumsum(cnt) - cnt
        for ci in range(p.NCH):
            n = int(cnt[ci])
            if n == 0:
                continue
            s0 = ci * p.TPC * 128
            e0 = lo + int(starts[ci])
            dsh[s0:s0 + n] = dijk[e0:e0 + n]
            sl = slice(int(starts[ci]), int(starts[ci]) + n)
            cidv[s0:s0 + n] = es[sl] % ACH
            idxv[s0:s0 + n] = ej[sl]

        dsh_T = np.ascontiguousarray(dsh.T)   # [n_in, E_PAD] bf16
        cid_tbl = np.ascontiguousarray(
            cidv[: p.T * 128].reshape(p.T, 128).T.astype(BF16)
        )                                      # [128, T]
        f_idx = _wrap_idx((idxv >> 1).reshape(p.NGC, GATH_E))
        par_tbl = np.ascontiguousarray(
            (idxv[: p.T * 128] & 1).reshape(p.T, 128).T.astype(BF16)
        )                                      # [128, T]

        per_core.append(
            dict(
                dijk_sh=dsh_T,
                cid=cid_tbl,
                par=par_tbl,
                f_idx=f_idx,
                xslice=_pad_rows(x[c * p.NA:(c + 1) * p.NA], p.NA_PAD),
            )
        )
    return per_core


def _wrap_idx(idx2d):
    """[ncalls, G] int -> [ncalls, 128, G//16] int16 SWDGE layout."""
    ncalls, g = idx2d.shape
    w = idx2d.astype(np.int16).reshape(ncalls, g // 16, 16)
    w = np.transpose(w, (0, 2, 1))            # [ncalls, 16, G//16]
    return np.ascontiguousarray(np.tile(w, (1, 8, 1)))


def _pad_rows(a, n):
    out = np.zeros((n,) + a.shape[1:], dtype=np.asarray(a).dtype)
    out[: a.shape[0]] = np.asarray(a)
    return out


def build_program(p):
    # Force one activation table (Exp+Ln coexist in natural_log_exp_and_others)
    # — otherwise bacc alternates per-func sets and pays 1.28us per switch.
    import concourse.bacc as _bacc_mod
    _orig_gat = _bacc_mod.get_activation_tables

    def _one_table(arch):
        t = _orig_gat(arch)
        keep = "natural_log_exp_and_others"
        assert keep in t
        return {k: (v if k == keep else set()) for k, v in t.items()}

    _bacc_mod.get_activation_tables = _one_table
    try:
        return _build_program_inner(p)
    finally:
        _bacc_mod.get_activation_tables = _orig_gat


def _build_program_inner(p):
    nc = bacc.Bacc(None, target_bir_lowering=False, num_swdge_queues=4)
    n_in = p.n_in

    # ---- dram parameters ----
    xT = nc.declare_dram_parameter("xT", [128, p.NX_PAD], dt.bfloat16, isOutput=False)
    xslice = nc.declare_dram_parameter("xslice", [p.NA_PAD, 128], dt.float32, isOutput=False)
    dijk_sh = nc.declare_dram_parameter("dijk_sh", [n_in, p.E_PAD], dt.bfloat16, isOutput=False)
    f_idx = nc.declare_dram_parameter("f_idx", [p.NGC, 128, GATH_E // 16], dt.int16, isOutput=False)
    cid = nc.declare_dram_parameter("cid", [128, p.T], dt.bfloat16, isOutput=False)
    par = nc.declare_dram_parameter("par", [128, p.T], dt.bfloat16, isOutput=False)
    w1b = nc.declare_dram_parameter("w1b", [n_in, 128], dt.bfloat16, isOutput=False)
    w2b = nc.declare_dram_parameter("w2b", [128, 128], dt.bfloat16, isOutput=False)
    winb = nc.declare_dram_parameter("winb", [128, 128], dt.bfloat16, isOutput=False)
    woutb = nc.declare_dram_parameter("woutb", [128, 128], dt.bfloat16, isOutput=False)
    wdb = nc.declare_dram_parameter("wdb", [128, 128], dt.bfloat16, isOutput=False)
    iota = nc.declare_dram_parameter("iota", [128, 4, 128], dt.bfloat16, isOutput=False)

    y_out = nc.declare_dram_parameter("y_out", [p.NA_PAD, 128], dt.float32, isOutput=True)
    v_out = nc.declare_dram_parameter("v_out", [p.NA_PAD, 128], dt.float32, isOutput=True)

    # ---- internal dram ----
    xf_dram = nc.dram_tensor("xf_dram", [p.NX_PAD, 128], dt.bfloat16)

    NKC = p.NFULL + (1 if p.KREM else 0)
    KC = [128] * p.NFULL + ([p.KREM] if p.KREM else [])

    with tile.TileContext(nc) as tc:
        with (
            tc.tile_pool(name="const", bufs=1) as constp,
            tc.tile_pool(name="xtp", bufs=2) as xtp,
            tc.tile_pool(name="xfp", bufs=2) as xfp,
            tc.tile_pool(name="dld", bufs=2) as dld,
            tc.tile_pool(name="fbp", bufs=8) as fbp,
            tc.tile_pool(name="eb", bufs=3) as eb,
            tc.tile_pool(name="sgp", bufs=2) as sgp,
            tc.tile_pool(name="tailp", bufs=2) as tailp,
            tc.tile_pool(name="psum", bufs=2, space="PSUM") as psum,
        ):
            # ---- constants ----
            w1sb = []
            for kc in range(NKC):
                kn = KC[kc]
                t = constp.tile([128, 128], dt.bfloat16, name=f"w1sb{kc}")
                nc.sync.dma_start(out=t[:kn, :], in_=w1b[kc * 128: kc * 128 + kn, :])
                w1sb.append(t)
            w2sb = constp.tile([128, 128], dt.bfloat16)
            nc.sync.dma_start(out=w2sb[:], in_=w2b[:, :])
            winsb = constp.tile([128, 128], dt.bfloat16)
            nc.sync.dma_start(out=winsb[:], in_=winb[:, :])
            woutsb = constp.tile([128, 128], dt.bfloat16)
            nc.sync.dma_start(out=woutsb[:], in_=woutb[:, :])
            wdsb = constp.tile([128, 128], dt.bfloat16)
            nc.sync.dma_start(out=wdsb[:], in_=wdb[:, :])
            iota4_sb = constp.tile([128, 4, 128], dt.bfloat16)
            nc.sync.dma_start(out=iota4_sb[:], in_=iota[:, :, :])
            cid_sb = constp.tile([128, p.T], dt.bfloat16)
            nc.sync.dma_start(out=cid_sb[:], in_=cid[:, :])
            par_sb = constp.tile([128, p.T], dt.bfloat16)
            nc.sync.dma_start(out=par_sb[:], in_=par[:, :])
            half_c = constp.tile([128, 1], dt.float32)
            nc.gpsimd.memset(half_c[:], 0.5)

            # ---- phase 0: xf = x @ Win -> xf_dram ----
            def write_xf(a0, xf_sb):
                nc.sync.dma_start(
                    out=xf_dram[a0:a0 + 512, :].rearrange(
                        "(j pp) f -> pp j f", pp=128
                    ),
                    in_=xf_sb[:],
                )

            xts = None
            for sg in range(p.NSG):
                if sg % 4 == 0:
                    wdt = min(2048, p.NX_PAD - sg * 512)
                    xts = xtp.tile([128, 2048], dt.bfloat16, tag="xts")
                    nc.sync.dma_start(
                        out=xts[:, :wdt],
                        in_=xT[:, sg * 512: sg * 512 + wdt],
                    )
                xf_ps = psum.tile([128, 4, 128], dt.float32, tag="z2")
                o = (sg % 4) * 512
                for jj in range(4):
                    nc.tensor.matmul(
                        xf_ps[:, jj, :],
                        xts[:, o + jj * 128: o + (jj + 1) * 128],
                        winsb[:],
                        start=True,
                        stop=True,
                    )
                xf_sb = xfp.tile([128, 4, 128], dt.bfloat16, tag="xfsb")
                nc.vector.tensor_copy(xf_sb[:], xf_ps[:])
                write_xf(sg * 512, xf_sb)

            # ---- tail emitter (per 128-atom chunk) ----
            def emit_tail(c, cps):
                cT = tailp.tile([128, 128], dt.bfloat16, tag="cT")
                nc.vector.tensor_copy(cT[:], cps[:])
                z3_ps = psum.tile([128, 128], dt.float32, tag="tail")
                nc.tensor.matmul(z3_ps[:], woutsb[:], cT[:], start=True, stop=True)
                e3 = tailp.tile([128, 128], dt.float32, tag="e3")
                nc.scalar.activation(e3[:], z3_ps[:], AF.Exp)
                hT = tailp.tile([128, 128], dt.bfloat16, tag="hT")
                nc.scalar.activation(
                    hT[:], e3[:], AF.Ln, bias=half_c[:], scale=half_c[:]
                )
                v_ps = psum.tile([128, 128], dt.float32, tag="tail")
                nc.tensor.matmul(v_ps[:], hT[:], wdsb[:], start=True, stop=True)
                v_sb = tailp.tile([128, 128], dt.float32, tag="v")
                nc.vector.tensor_copy(v_sb[:], v_ps[:])
                nc.sync.dma_start(
                    out=v_out[c * 128:(c + 1) * 128, :], in_=v_sb[:]
                )
                xs = tailp.tile([128, 128], dt.float32, tag="xs")
                nc.sync.dma_start(
                    out=xs[:], in_=xslice[c * 128:(c + 1) * 128, :]
                )
                y_sb = tailp.tile([128, 128], dt.float32, tag="y")
                nc.vector.tensor_tensor(y_sb[:], v_sb[:], xs[:], ALU.add)
                nc.sync.dma_start(
                    out=y_out[c * 128:(c + 1) * 128, :], in_=y_sb[:]
                )

            # ---- phase A: edge pipeline ----
            cur_cps = None
            fsum = None
            for q in range(p.NQUAD):
                e0 = q * LOAD_E
                dq = None
                if p.NFULL:
                    dq = dld.tile([128, p.NFULL, LOAD_E], dt.bfloat16, tag="dq")
                    nc.sync.dma_start(
                        out=dq[:],
                        in_=dijk_sh[0:p.NFULL * 128, e0:e0 + LOAD_E].rearrange(
                            "(s pp) e -> pp s e", pp=128
                        ),
                    )
                d44 = None
                if p.KREM:
                    d44 = dld.tile([p.KREM, LOAD_E], dt.bfloat16, tag="d44")
                    nc.sync.dma_start(
                        out=d44[:],
                        in_=dijk_sh[p.NFULL * 128:n_in, e0:e0 + LOAD_E],
                    )
                fgats = []
                for L in range(LOAD_E // GATH_E):
                    call = q * (LOAD_E // GATH_E) + L
                    idxt = fbp.tile([128, GATH_E // 16], dt.int16, tag="idx")
                    nc.sync.dma_start(out=idxt[:], in_=f_idx[call, :, :])
                    fgat = fbp.tile([128, GATH_E // 128, 256], dt.bfloat16, tag="fgat")
                    nc.gpsimd.dma_gather(
                        fgat[:],
                        xf_dram[:, :].rearrange("(r two) f -> r (two f)", two=2),
                        idxt[:], GATH_E, GATH_E, 256,
                        single_packet=True, queue_num=call % 4,
                    )
                    fgats.append(fgat)

                nblk_load = LOAD_E // 512
                for grp in range(nblk_load // 4):
                    b0 = q * nblk_load + grp * 4
                    if b0 * 4 >= p.T:
                        break
                    nqb = min(4, _ceil(p.T - b0 * 4, 4))
                    full_quad = (b0 + 4) * 4 <= p.T
                    e1q = eb.tile([128, 4, 512], dt.float32, tag="e1")
                    t1sq = eb.tile([128, 4, 512], dt.bfloat16, tag="t1s")
                    e2q = eb.tile([128, 4, 512], dt.float32, tag="e2")
                    wq = eb.tile([128, 4, 512], dt.bfloat16, tag="w")
                    # --- stage 1: mm1 + Exp per block, Ln batched ---
                    for sb in range(nqb):
                        b = b0 + sb
                        lsb = grp * 4 + sb
                        t0 = b * 4
                        ntile = min(4, p.T - t0)
                        ne = ntile * 128
                        t1_ps = psum.tile([128, 512], dt.float32, tag="t1")
                        for kc in range(NKC):
                            kn = KC[kc]
                            srcm = dq[:, kc, lsb * 512: lsb * 512 + ne] if kc < p.NFULL \
                                else d44[:kn, lsb * 512: lsb * 512 + ne]
                            nc.tensor.matmul(
                                t1_ps[:, :ne], w1sb[kc][:kn, :], srcm,
                                start=(kc == 0), stop=(kc == NKC - 1),
                            )
                        nc.scalar.activation(e1q[:, sb, :ne], t1_ps[:, :ne], AF.Exp)
                        if not full_quad:
                            nc.scalar.activation(
                                t1sq[:, sb, :ne], e1q[:, sb, :ne], AF.Ln,
                                bias=half_c[:], scale=half_c[:],
                            )
                    if full_quad:
                        nc.scalar.activation(
                            t1sq[:], e1q[:], AF.Ln, bias=half_c[:], scale=half_c[:]
                        )
                    # --- stage 2: mm2 + Exp per block, Ln batched ---
                    for sb in range(nqb):
                        b = b0 + sb
                        t0 = b * 4
                        ntile = min(4, p.T - t0)
                        ne = ntile * 128
                        z2_ps = psum.tile([128, 4, 128], dt.float32, tag="z2")
                        for i in range(ntile):
                            nc.tensor.matmul(
                                z2_ps[:, i, :],
                                t1sq[:, sb, i * 128:(i + 1) * 128],
                                w2sb[:],
                                start=True, stop=True,
                            )
                        nc.scalar.activation(
                            e2q[:, sb, :ne],
                            z2_ps[:, :ntile, :].rearrange("pp i f -> pp (i f)"),
                            AF.Exp,
                        )
                        if not full_quad:
                            nc.scalar.activation(
                                wq[:, sb, :ne], e2q[:, sb, :ne], AF.Ln,
                                bias=half_c[:], scale=half_c[:],
                            )
                    if full_quad:
                        nc.scalar.activation(
                            wq[:], e2q[:], AF.Ln, bias=half_c[:], scale=half_c[:]
                        )
                    # --- stage 3: select f, wf, S one-hot, conv accumulation ---
                    for sb in range(nqb):
                        b = b0 + sb
                        lsb = grp * 4 + sb
                        t0 = b * 4
                        ntile = min(4, p.T - t0)
                        fgc = fgats[lsb // 2]
                        fo = (lsb % 2) * 4
                        fL = fgc[:, fo:fo + ntile, 0:128]
                        fR = fgc[:, fo:fo + ntile, 128:256]
                        dR = sgp.tile([128, 4, 128], dt.bfloat16, tag="dR")
                        nc.vector.tensor_tensor(dR[:, :ntile, :], fR, fL, ALU.subtract)
                        pd = sgp.tile([128, 4, 128], dt.bfloat16, tag="pd")
                        nc.vector.tensor_tensor(
                            pd[:, :ntile, :], dR[:, :ntile, :],
                            par_sb[:, t0:t0 + ntile].to_broadcast([128, ntile, 128]),
                            ALU.mult,
                        )
                        fsel = sgp.tile([128, 4, 128], dt.bfloat16, tag="fsel")
                        nc.vector.tensor_tensor(fsel[:, :ntile, :], pd[:, :ntile, :], fL, ALU.add)
                        wf = sgp.tile([128, 4, 128], dt.bfloat16, tag="wf")
                        nc.vector.tensor_tensor(
                            wf[:, :ntile, :],
                            wq[:, sb, : ntile * 128].rearrange("pp (i f) -> pp i f", i=ntile),
                            fsel[:, :ntile, :],
                            ALU.mult,
                        )
                        S_blk = sgp.tile([128, 4, 128], dt.bfloat16, tag="S")
                        nc.vector.tensor_tensor(
                            S_blk[:, :ntile, :],
                            iota4_sb[:, :ntile, :],
                            cid_sb[:, t0:t0 + ntile].to_broadcast([128, ntile, 128]),
                            ALU.is_equal,
                        )
                        for i in range(ntile):
                            t = t0 + i
                            c = t // p.TPC
                            k = t % p.TPC
                            if k == 0:
                                cur_cps = psum.tile([128, 128], dt.float32, tag="conv")
                            nc.tensor.matmul(
                                cur_cps[:], wf[:, i, :], S_blk[:, i, :],
                                start=(k == 0), stop=(k == p.TPC - 1),
                            )
                            if k == p.TPC - 1:
                                emit_tail(c, cur_cps)

    nc.finalize()
    return nc


_PROG_CACHE = {}


def _compute_tpc(n_atoms, seg_i):
    na = n_atoms // N_CORES
    seg_i = np.asarray(seg_i).astype(np.int64)
    bounds = np.searchsorted(seg_i, np.arange(N_CORES + 1) * na)
    tpc = 1
    nch = _ceil(na, ACH)
    for c in range(N_CORES):
        es = seg_i[bounds[c]:bounds[c + 1]] - c * na
        if len(es) == 0:
            continue
        cnt = np.bincount(es // ACH, minlength=nch)
        tpc = max(tpc, _ceil(int(cnt.max()), 128))
    return tpc


def kernel(x, dijk, W1, b1, W2, b2, Win, Wout, bout, Wd, bd, idx_j, seg_i, seg_j):
    x = np.ascontiguousarray(np.asarray(x, dtype=np.float32))
    dijk = np.ascontiguousarray(np.asarray(dijk, dtype=np.float32))
    for b in (b1, b2, bout, bd):
        assert np.abs(np.asarray(b)).max() == 0.0, "nonzero biases unsupported"

    n_atoms, n_basis = x.shape
    n_edges, n_in = dijk.shape
    assert n_basis == 128 and np.asarray(W2).shape == (128, 128)

    tpc = _compute_tpc(n_atoms, seg_i)
    p = Plan(n_atoms, n_edges, n_in, tpc)
    per_core = shard_inputs(p, x, dijk, idx_j, seg_i)

    key = (n_atoms, n_edges, n_in, tpc)
    if key not in _PROG_CACHE:
        _PROG_CACHE[key] = build_program(p)
    nc = _PROG_CACHE[key]

    xTh = np.zeros((128, p.NX_PAD), dtype=BF16)
    xTh[:, :n_atoms] = x.T
    common = dict(
        xT=xTh,
        w1b=np.asarray(W1, dtype=np.float32).astype(BF16),
        w2b=np.asarray(W2, dtype=np.float32).astype(BF16),
        winb=np.asarray(Win, dtype=np.float32).astype(BF16),
        woutb=np.asarray(Wout, dtype=np.float32).astype(BF16),
        wdb=np.asarray(Wd, dtype=np.float32).astype(BF16),
        iota=np.tile(np.arange(128, dtype=np.float32).astype(BF16), (128, 4, 1)),
    )
    in_maps = [{**common, **pc} for pc in per_core]
    res = run_bass_kernel_spmd(nc, in_maps, list(range(N_CORES)))
    global LAST_RESULTS
    LAST_RESULTS = res

    y = np.empty((n_atoms, 128), dtype=np.float32)
    v = np.empty((n_atoms, 128), dtype=np.float32)
    for c in range(N_CORES):
        y[c * p.NA:(c + 1) * p.NA] = res.results[c]["y_out"][: p.NA]
        v[c * p.NA:(c + 1) * p.NA] = res.results[c]["v_out"][: p.NA]
    return (y, v)

